# revision 1
# baseline (speedup 1.0000x reference)
"""Multi-head self-attention (RoPE, causal) on 8 Trainium2 NeuronCores.

Sharding: tensor-parallel over heads. Each core owns 2 of 16 heads:
  - QKV projections column-sharded (each core computes its 128 features)
  - attention per (batch, head) pair fully on-core, scores kept in the
    transposed orientation [tk, tq] so softmax needs no PE transposes:
    exp on ACT, denominator via a ones-row appended to V, causal handled
    block-wise + a triangular mask on diagonal blocks
  - AllToAll switches from head-sharding to token-sharding (4MB/core)
  - output projection token-sharded, output written in natural layout

dtypes: fp32r (TF32-like, full PE rate at N>=512) everywhere except the
softmax probabilities / V operand of the PV matmul, which are bf16.
"""

import numpy as np
import ml_dtypes

import concourse.bacc as bacc
import concourse.mybir as mybir
import concourse.tile as tile
from concourse import bass_utils
from concourse.masks import make_identity

F32 = mybir.dt.float32
F32R = mybir.dt.float32r
BF16 = mybir.dt.bfloat16

B, T, D = 4, 2048, 1024
H, DH = 16, 64
N_CORES = 8
HPC = H // N_CORES            # heads per core = 2
EC = HPC * DH                 # feature slice per core = 128
NT = B * T                    # 8192 tokens
TPC = NT // N_CORES           # 1024 tokens per core
THETA = 10000.0

_CACHE = {}
last_results = None  # BassKernelResults of the most recent run (for profiling)


def _build_program():
    nc = bacc.Bacc("TRN2", debug=False, target_bir_lowering=False,
                   num_devices=N_CORES)

    xt_d = nc.dram_tensor("xt", [128, 8, NT], BF16, kind="ExternalInput")
    wq_d = nc.dram_tensor("wq", [128, 8, EC], BF16, kind="ExternalInput")
    wk_d = nc.dram_tensor("wk", [128, 8, EC], BF16, kind="ExternalInput")
    wv_d = nc.dram_tensor("wv", [128, 8, EC], BF16, kind="ExternalInput")
    wo_d = nc.dram_tensor("wo", [128, 8, D], BF16, kind="ExternalInput")
    cos_d = nc.dram_tensor("cosb", [128, T], F32, kind="ExternalInput")
    sin_d = nc.dram_tensor("sinb", [128, T], F32, kind="ExternalInput")
    rotm_d = nc.dram_tensor("rotm", [128, 128], F32R, kind="ExternalInput")
    tri_d = nc.dram_tensor("trimask", [128, 128], BF16, kind="ExternalInput")
    y_d = nc.dram_tensor("y", [TPC, D], F32, kind="ExternalOutput")

    NB = T // 128      # 16 tk blocks per batch
    NCHUNK = NT // 512  # 16 phase-1 chunks

    with tile.TileContext(nc) as tc:
        with (
            tc.tile_pool(name="consts", bufs=1) as consts,
            tc.tile_pool(name="wpool", bufs=1) as wpool,
            tc.tile_pool(name="big", bufs=1) as big,
            tc.tile_pool(name="xp", bufs=2) as xp,
            tc.tile_pool(name="stage", bufs=2) as stage,
            tc.tile_pool(name="expp", bufs=4) as expp,
            tc.tile_pool(name="outp", bufs=2) as outp,
            tc.tile_pool(name="psA", bufs=1, space="PSUM") as psA,
            tc.tile_pool(name="psB", bufs=1, space="PSUM") as psB,
            tc.tile_pool(name="pvA", bufs=2, space="PSUM") as pvA,
            tc.tile_pool(name="pvB", bufs=2, space="PSUM") as pvB,
            tc.tile_pool(name="dram", bufs=2, space="DRAM") as dram,
        ):
            # ---- constants ----
            cos_sb = consts.tile([128, T], F32)
            sin_sb = consts.tile([128, T], F32)
            rotm_sb = consts.tile([128, 128], F32R)
            tri_sb = consts.tile([128, 128], BF16)
            ident_sb = consts.tile([128, 128], F32)
            nc.sync.dma_start(cos_sb[:], cos_d[:, :])
            nc.sync.dma_start(sin_sb[:], sin_d[:, :])
            nc.sync.dma_start(rotm_sb[:], rotm_d[:, :])
            nc.sync.dma_start(tri_sb[:], tri_d[:, :])
            make_identity(nc, ident_sb[:])

            wq_sb = consts.tile([128, 8, EC], BF16)
            wk_sb = consts.tile([128, 8, EC], BF16)
            wv_sb = consts.tile([128, 8, EC], BF16)
            nc.sync.dma_start(wq_sb[:], wq_d[:, :, :])
            nc.sync.dma_start(wk_sb[:], wk_d[:, :, :])
            nc.sync.dma_start(wv_sb[:], wv_d[:, :, :])

            # ---- persistent tensors ----
            qT = big.tile([128, NT], F32R, tag="qT")
            kT = big.tile([128, NT], F32R, tag="kT")
            # V per (pair, tk-block): [tk=128, 65] with ones in col 64
            vext = big.tile([128, HPC * B, NB, 65], BF16, tag="vext")
            nc.vector.memset(vext[:, :, :, 64], 1.0)

            a2a_in1 = dram.tile([N_CORES, 128, 768], BF16)
            a2a_out1 = dram.tile([N_CORES, 128, 768], BF16)
            a2a_in2 = dram.tile([N_CORES, 128, 256], BF16)
            a2a_out2 = dram.tile([N_CORES, 128, 256], BF16)

            # ================= Phase 1: QKV projections + RoPE =============
            def do_chunk(ci):
                t0 = 512 * ci
                bb = t0 // T
                s0 = t0 % T
                xt = xp.tile([128, 8, 512], BF16, tag="x")
                nc.sync.dma_start(xt[:], xt_d[:, :, t0:t0 + 512])

                # pipelined: proj(q) -> ACTcopy(q) -> proj(k) -> rot(q) ->
                # ACTcopy(k) -> proj(v) -> rot(k) -> ACTcopy(v) -> vtrans
                # so PE never sits behind an ACT drain.
                def _proj(w_sb, nm, pool):
                    pt = pool.tile([128, 1024], F32, tag="s", name="p" + nm)
                    pt = pt[:, 0:512]
                    for ko in range(8):
                        nc.tensor.matmul(pt, w_sb[:, ko, :], xt[:, ko, :],
                                         start=(ko == 0), stop=(ko == 7))
                    return pt

                def _rot(raw, nm, pool):
                    rot = pool.tile([128, 1024], F32, tag="s", name="r" + nm)
                    rot = rot[:, 0:512]
                    nc.tensor.matmul(rot, rotm_sb[:], raw[:],
                                     start=True, stop=True)
                    return rot

                def _rope_combine(raw, rot, dest):
                    t1 = stage.tile([128, 512], F32, tag="t1")
                    nc.vector.tensor_tensor(
                        t1[:], raw[:], cos_sb[:, s0:s0 + 512],
                        mybir.AluOpType.mult)
                    t2 = stage.tile([128, 512], F32, tag="t2")
                    nc.vector.tensor_tensor(
                        t2[:], rot[:], sin_sb[:, s0:s0 + 512],
                        mybir.AluOpType.mult)
                    nc.vector.tensor_tensor(
                        dest[:, t0:t0 + 512], t1[:], t2[:],
                        mybir.AluOpType.add)

                pq = _proj(wq_sb, "q", psA)
                rawq = stage.tile([128, 512], F32R, tag="rawq")
                nc.scalar.copy(rawq[:], pq)
                pk = _proj(wk_sb, "k", psB)
                rotq = _rot(rawq, "q", psA)
                rawk = stage.tile([128, 512], F32R, tag="rawk")
                nc.scalar.copy(rawk[:], pk)
                pv_ = _proj(wv_sb, "v", psB)
                rotk = _rot(rawk, "k", psA)
                vraw = stage.tile([128, 512], F32, tag="vraw")
                nc.scalar.copy(vraw[:], pv_)
                _rope_combine(rawq, rotq, qT)
                _rope_combine(rawk, rotk, kT)
                for h in range(HPC):
                    pair = bb * HPC + h
                    for bi in range(4):
                        jg = s0 // 128 + bi
                        tp = (psB if bi % 2 else psA).tile(
                            [128, 1024], F32, tag="s", name="vtr")[:, 0:64]
                        nc.tensor.transpose(
                            tp,
                            vraw[64 * h:64 * h + 64,
                                 128 * bi:128 * bi + 128],
                            ident_sb[64 * h:64 * h + 64,
                                     64 * h:64 * h + 64])
                        nc.vector.tensor_copy(
                            vext[:, pair, jg, 0:64], tp)

            # ================= Phase 2: attention ==========================
            # Two heads of the same batch run as interleaved pipeline
            # streams: ACT-exp latency of one stream hides behind PE work
            # of the other.
            def do_attn(bb):
                tb0 = bb * T
                qs = [qT[64 * hh:64 * hh + 64, tb0:tb0 + T] for hh in range(2)]
                ks = [kT[64 * hh:64 * hh + 64, tb0:tb0 + T] for hh in range(2)]
                spools = [psA, psB]
                vpools = [pvA, pvB]
                for c2 in range(2):
                    jmax = 8 * (c2 + 1)
                    pvt = [[vpools[hh].tile(
                        [65, 512], F32, tag="pv",
                        name=f"pv_{bb}_{hh}_{c2}_{hf}") for hf in range(2)]
                        for hh in range(2)]

                    def _scores_pair(j):
                        # both heads' score matmuls, issued alternating so
                        # the two K=64 row-strips (partitions 0-63 / 64-127)
                        # execute concurrently in the PE array.
                        spts = [spools[hh].tile(
                            [128, 1024], F32, tag="s",
                            name=f"s_{bb}_{hh}_{c2}_{j}") for hh in range(2)]
                        for hf in range(2):
                            cl0 = 1024 * c2 + 512 * hf
                            if cl0 + 512 <= 128 * j:
                                continue
                            w = cl0 + 512 - max(cl0, 128 * j)
                            N = 512 if w == 512 else max(256, w)
                            st = cl0 + 512 - N
                            for hh in range(2):
                                nc.tensor.matmul(
                                    spts[hh][:, st - 1024 * c2:
                                             st - 1024 * c2 + N],
                                    ks[hh][:, 128 * j:128 * j + 128],
                                    qs[hh][:, st:st + N],
                                    start=True, stop=True)
                        return spts

                    def _pv(j, exs):
                        lo = max(0, 128 * j - 1024 * c2)
                        for hh in range(2):
                            vt = vext[:, bb * HPC + hh, j, :]
                            for hf in range(2):
                                h0 = 512 * hf
                                a = max(h0, lo)
                                if a < h0 + 512:
                                    last_j = min(jmax - 1,
                                                 8 * c2 + 4 * hf + 3)
                                    nc.tensor.matmul(
                                        pvt[hh][hf][:, a - h0:512],
                                        vt, exs[hh][:, a:h0 + 512],
                                        start=(j == 0), stop=(j == last_j))

                    # software pipeline with one-iteration PV delay: the PE
                    # always has ready work (PV of j-1) at its queue head
                    # while ACT computes exp(j).
                    spt = _scores_pair(0)
                    prev = None
                    for j in range(jmax):
                        lo = max(0, 128 * j - 1024 * c2)
                        exs = []
                        for hh in range(2):
                            ex = expp.tile([128, 1024], BF16, tag="e",
                                           name=f"e_{hh}")
                            nc.scalar.activation(
                                ex[:, lo:1024], spt[hh][:, lo:1024],
                                mybir.ActivationFunctionType.Exp, scale=0.125)
                            exs.append(ex)
                        if prev is not None:
                            _pv(prev[0], prev[1])
                        if j + 1 < jmax:
                            spt = _scores_pair(j + 1)
                        for hh in range(2):
                            if 128 * j >= 1024 * c2:
                                nc.vector.tensor_tensor(
                                    exs[hh][:, lo:lo + 128],
                                    exs[hh][:, lo:lo + 128],
                                    tri_sb[:], mybir.AluOpType.mult)
                        prev = (j, exs)
                    _pv(prev[0], prev[1])
                    # normalize + ship to a2a_in.  Copy psum out first
                    # (ACT) so the pv slots free up for the next chunk.
                    for hh in range(2):
                        dnm = outp.tile([33, 512], F32, tag="dnm")
                        unn = [None, None]
                        for hf in range(2):
                            nc.vector.tensor_copy(
                                dnm[32 * hf:32 * hf + 1, :],
                                pvt[hh][hf][64:65, :])
                            unn[hf] = outp.tile([64, 512], BF16,
                                                tag=f"unn{hf}",
                                                name=f"unn{hf}")
                            nc.scalar.copy(unn[hf][:], pvt[hh][hf][0:64, :])
                        rec = outp.tile([33, 512], F32, tag="rec")
                        nc.vector.reciprocal(rec[:], dnm[:])
                        rscr = dram.tile([2, 512], F32, tag="rscr",
                                         name="rscr")
                        for hf in range(2):
                            nc.sync.dma_start(rscr[hf:hf + 1, :],
                                              rec[32 * hf:32 * hf + 1, :])
                        for hf in range(2):
                            recb = outp.tile([64, 512], F32, tag="recb")
                            nc.sync.dma_start(
                                recb[:],
                                rscr[hf:hf + 1, :].to_broadcast((64, 512)))
                            ao = outp.tile([64, 512], BF16, tag="ao")
                            nc.vector.tensor_tensor(
                                ao[:], unn[hf][:], recb[:],
                                mybir.AluOpType.mult)
                            # group 1 = batches 0-2 (768 tokens/dest),
                            # group 2 = batch 3 (256 tokens/dest)
                            if bb < 3:
                                grp, base, W = a2a_in1, 0, 768
                            else:
                                grp, base, W = a2a_in2, 6144, 256
                            tt = 2048 * bb + 1024 * c2 + 512 * hf - base
                            off = 0
                            while off < 512:
                                dd = (tt + off) // W
                                col = (tt + off) % W
                                w = min(512 - off, W - col)
                                nc.sync.dma_start(
                                    grp[dd, 64 * hh:64 * hh + 64,
                                        col:col + w],
                                    ao[:, off:off + w])
                                off += w

            def do_oproj(g, oall_g, row0, ntb):
                # y rows [row0, row0 + 128*ntb) from group-g tokens
                for eo in range(2):
                    wo_sb = wpool.tile([128, 8, 512], BF16, tag="wo",
                                       name=f"wo_{g}_{eo}")
                    nc.sync.dma_start(wo_sb[:],
                                      wo_d[:, :, 512 * eo:512 * eo + 512])
                    for tb in range(ntb):
                        ot = (psB if (tb + eo) % 2 else psA).tile(
                            [128, 1024], F32, tag="s", name="ot")[:, 0:512]
                        for ec in range(8):
                            nc.tensor.matmul(
                                ot, oall_g[:, ec, 128 * tb:128 * tb + 128],
                                wo_sb[:, ec, :],
                                start=(ec == 0), stop=(ec == 7))
                        ys = outp.tile([128, 512], F32, tag="y")
                        nc.scalar.copy(ys[:], ot)
                        nc.sync.dma_start(
                            y_d[row0 + 128 * tb:row0 + 128 * tb + 128,
                                512 * eo:512 * eo + 512], ys[:])

            # interleave phase 1 and attention per batch; group-0 A2A and
            # its output projection overlap batches 2-3.
            rg = [list(range(N_CORES))]
            for bb in range(3):
                for ci in range(4 * bb, 4 * bb + 4):
                    do_chunk(ci)
                do_attn(bb)
            nc.gpsimd.collective_compute(
                "AllToAll", mybir.AluOpType.bypass, replica_groups=rg,
                ins=[a2a_in1.opt()], outs=[a2a_out1.opt()])
            oall1 = wpool.tile([128, 8, 768], BF16, tag="oall1")
            nc.sync.dma_start(oall1[:],
                              a2a_out1[:].rearrange("s p t -> p s t"))
            for ci in range(12, 16):
                do_chunk(ci)
            do_attn(3)
            nc.gpsimd.collective_compute(
                "AllToAll", mybir.AluOpType.bypass, replica_groups=rg,
                ins=[a2a_in2.opt()], outs=[a2a_out2.opt()])
            do_oproj(0, oall1, 0, 6)
            oall2 = wpool.tile([128, 8, 256], BF16, tag="oall2")
            nc.sync.dma_start(oall2[:],
                              a2a_out2[:].rearrange("s p t -> p s t"))
            do_oproj(1, oall2, 768, 2)

    nc.compile()
    return nc


def _host_inputs(x, Wq, Wk, Wv, Wo, token_positions):
    """Per-core in_maps with transposed/tiled layouts."""
    x = np.asarray(x, dtype=np.float32)
    xt_bf = np.ascontiguousarray(
        x.reshape(NT, D).T.reshape(8, 128, NT).transpose(1, 0, 2)
    ).astype(ml_dtypes.bfloat16)

    pos = np.asarray(token_positions).astype(np.float64)
    inv_freq = 1.0 / (THETA ** (np.arange(0, DH, 2, dtype=np.float64) / DH))
    ang = pos[None, :] * inv_freq[:, None]          # [32, T]
    cos_p = np.cos(ang)                              # pair i
    sin_p = np.sin(ang)
    # partition p (0..127): within-head dim d = p % 64, pair = d // 2
    d_idx = (np.arange(128) % 64) // 2
    cosb = cos_p[d_idx, :].astype(np.float32)
    sinb = sin_p[d_idx, :].astype(np.float32)

    rotm = np.zeros((128, 128), dtype=np.float32)
    for i in range(64):
        rotm[2 * i + 1, 2 * i] = -1.0   # out[2i] -= in[2i+1]*sin -> rot[2i] = -in[2i+1]
        rotm[2 * i, 2 * i + 1] = 1.0    # rot[2i+1] = in[2i]
    tri = np.tril(np.ones((128, 128), dtype=np.float32)).T  # [tk, tq] tk<=tq
    tri = tri.astype(ml_dtypes.bfloat16)

    def wtiles(W, sl):
        # lhsT tiles: [p, ko, e] with d = ko*128+p contracting
        Wt = np.ascontiguousarray(W[sl, :].T)        # [D, e]
        return np.ascontiguousarray(
            Wt.reshape(8, 128, Wt.shape[1]).transpose(1, 0, 2))

    WoT = np.ascontiguousarray(np.asarray(Wo, dtype=np.float32).T)  # [e_in, e_out]
    wo_t = np.ascontiguousarray(WoT.reshape(8, 128, D).transpose(1, 0, 2))

    in_maps = []
    for c in range(N_CORES):
        sl = slice(EC * c, EC * (c + 1))
        in_maps.append({
            "xt": xt_bf,
            "wq": wtiles(np.asarray(Wq, np.float32), sl).astype(ml_dtypes.bfloat16),
            "wk": wtiles(np.asarray(Wk, np.float32), sl).astype(ml_dtypes.bfloat16),
            "wv": wtiles(np.asarray(Wv, np.float32), sl).astype(ml_dtypes.bfloat16),
            "wo": wo_t.astype(ml_dtypes.bfloat16),
            "cosb": cosb,
            "sinb": sinb,
            "rotm": rotm,
            "trimask": tri,
        })
    return in_maps


def kernel(x, Wq, Wk, Wv, Wo, token_positions):
    global last_results
    if "nc" not in _CACHE:
        _CACHE["nc"] = _build_program()
    nc = _CACHE["nc"]
    in_maps = _host_inputs(x, Wq, Wk, Wv, Wo, token_positions)
    res = bass_utils.run_bass_kernel_spmd(nc, in_maps, list(range(N_CORES)))
    last_results = res
    y = np.empty((NT, D), dtype=np.float32)
    for c in range(N_CORES):
        yc = res.results[c]["y"]
        y[768 * c:768 * c + 768] = yc[0:768]
        y[6144 + 256 * c:6144 + 256 * c + 256] = yc[768:1024]
    return y.reshape(B, T, D)



# revision 8
# speedup vs baseline: 1.0768x; 1.0768x over previous
"""Multi-head self-attention (RoPE, causal) on 8 Trainium2 NeuronCores.

Sharding: tensor-parallel over heads. Each core owns 2 of 16 heads:
  - QKV projections column-sharded (each core computes its 128 features)
  - V^T computed directly (x-block stationary) so no PE transposes
  - attention per (batch, head) pair on-core, scores kept transposed
    [tk, tq]; the two heads' K=64 score matmuls run concurrently in the
    PE array (row tiling via base_partition 0/64); softmax denominator
    via a ones-column appended to V^T
  - per-batch AllToAll switches from head- to token-sharding; the output
    projection for batch b overlaps attention of batch b+1
  - emission order interleaves proj(b+1) and oproj(b-1) into attn(b) so
    the PE never idles past the HAM window (stays at 2.4 GHz)

dtypes: bf16 operands everywhere (FWL stays enabled), fp32 PSUM accum.
"""

import numpy as np
import ml_dtypes

import concourse.bacc as bacc
import concourse.mybir as mybir
import concourse.tile as tile
from concourse import bass_utils

F32 = mybir.dt.float32
BF16 = mybir.dt.bfloat16

B, T, D = 4, 2048, 1024
H, DH = 16, 64
N_CORES = 8
HPC = H // N_CORES            # heads per core = 2
EC = HPC * DH                 # feature slice per core = 128
NT = B * T                    # 8192 tokens
THETA = 10000.0
NB = T // 128                 # 16 tk blocks per batch
TPB = T // N_CORES            # 256 tokens per (core, batch) after A2A

_CACHE = {}
last_results = None


def _build_program():
    nc = bacc.Bacc("TRN2", debug=False, target_bir_lowering=False,
                   num_devices=N_CORES)

    xt_d = nc.dram_tensor("xt", [128, 8, NT], BF16, kind="ExternalInput")
    wq_d = nc.dram_tensor("wq", [128, 8, EC], BF16, kind="ExternalInput")
    wk_d = nc.dram_tensor("wk", [128, 8, EC], BF16, kind="ExternalInput")
    wv_d = nc.dram_tensor("wv", [128, 8, EC], BF16, kind="ExternalInput")
    wo_d = nc.dram_tensor("wo", [128, 8, D], BF16, kind="ExternalInput")
    cos_d = nc.dram_tensor("cosb", [128, T], F32, kind="ExternalInput")
    sin_d = nc.dram_tensor("sinb", [128, T], F32, kind="ExternalInput")
    rotm_d = nc.dram_tensor("rotm", [128, 128], BF16, kind="ExternalInput")
    tri_d = nc.dram_tensor("trimask", [128, 128], BF16, kind="ExternalInput")
    y_d = nc.dram_tensor("y", [B * TPB, D], F32, kind="ExternalOutput")

    with tile.TileContext(nc) as tc:
        with (
            tc.tile_pool(name="consts", bufs=1) as consts,
            tc.tile_pool(name="big", bufs=1) as big,
            tc.tile_pool(name="xp", bufs=2) as xp,
            tc.tile_pool(name="stage", bufs=2) as stage,
            tc.tile_pool(name="expp", bufs=6) as expp,
            tc.tile_pool(name="outp", bufs=3) as outp,
            tc.tile_pool(name="oall", bufs=2) as oallp,
            tc.tile_pool(name="pp", bufs=5, space="PSUM") as pp,
            tc.tile_pool(name="pv", bufs=3, space="PSUM") as pvp,
            tc.tile_pool(name="dram", bufs=1, space="DRAM") as dram,
        ):
            # ---- constants ----
            cos_sb = consts.tile([128, T], F32)
            sin_sb = consts.tile([128, T], F32)
            rotm_sb = consts.tile([128, 128], BF16)
            tri_sb = consts.tile([128, 128], BF16)
            nc.sync.dma_start(cos_sb[:], cos_d[:, :])
            nc.sync.dma_start(sin_sb[:], sin_d[:, :])
            nc.sync.dma_start(rotm_sb[:], rotm_d[:, :])
            nc.sync.dma_start(tri_sb[:], tri_d[:, :])

            wq_sb = consts.tile([128, 8, EC], BF16)
            wk_sb = consts.tile([128, 8, EC], BF16)
            wv_sb = consts.tile([128, 8, EC], BF16)
            wo_sb = consts.tile([128, 8, D], BF16)
            nc.sync.dma_start(wq_sb[:], wq_d[:, :, :])
            nc.sync.dma_start(wk_sb[:], wk_d[:, :, :])
            nc.sync.dma_start(wv_sb[:], wv_d[:, :, :])
            nc.sync.dma_start(wo_sb[:], wo_d[:, :, :])

            # ---- persistent tensors ----
            qT = big.tile([128, NT], BF16, tag="qT")
            kT = big.tile([128, NT], BF16, tag="kT")
            # V^T per (pair, tk-block): [tk=128, 128] cols 0-63 = v,
            # col 64 = ones (softmax denominator), cols 65-127 unused
            # (full-128 stationary keeps FWL enabled on the PV matmuls).
            vext = big.tile([128, HPC * B, NB, 128], BF16, tag="vext")
            nc.vector.memset(vext[:, :, :, 64], 1.0)

            a2a_in = [dram.tile([N_CORES, 128, TPB], BF16, tag=f"a2ai{b}",
                                name=f"a2ai{b}") for b in range(B)]
            a2a_out = [dram.tile([N_CORES, 128, TPB], BF16, tag=f"a2ao{b}",
                                 name=f"a2ao{b}") for b in range(B)]
            rg = [list(range(N_CORES))]

            # ============ QKV projections + RoPE for one 512-token chunk
            def proj_chunk(ci):
                t0 = 512 * ci
                bb = t0 // T
                xt = xp.tile([128, 8, 512], BF16, tag="x")
                nc.sync.dma_start(xt[:], xt_d[:, :, t0:t0 + 512])

                def _proj(w_sb, nm):
                    pt = pp.tile([128, 512], F32, tag="pp", name="p" + nm)
                    for ko in range(8):
                        nc.tensor.matmul(pt, w_sb[:, ko, :], xt[:, ko, :],
                                         start=(ko == 0), stop=(ko == 7))
                    return pt

                def _rope(raw_ps, nm):
                    raw = stage.tile([128, 512], BF16, tag="raw" + nm)
                    nc.vector.tensor_copy(raw[:], raw_ps)
                    rot = pp.tile([128, 512], F32, tag="pp", name="r" + nm)
                    nc.tensor.matmul(rot, rotm_sb[:], raw[:],
                                     start=True, stop=True)
                    return raw, rot

                def _combine(raw, rot, dest, nm):
                    s0 = t0 % T
                    t1 = stage.tile([128, 512], F32, tag="t1" + nm)
                    nc.vector.tensor_tensor(
                        t1[:], raw[:], cos_sb[:, s0:s0 + 512],
                        mybir.AluOpType.mult)
                    t2 = stage.tile([128, 512], F32, tag="t2" + nm)
                    nc.vector.tensor_tensor(
                        t2[:], rot[:], sin_sb[:, s0:s0 + 512],
                        mybir.AluOpType.mult)
                    nc.vector.tensor_tensor(
                        dest[:, t0:t0 + 512], t1[:], t2[:],
                        mybir.AluOpType.add)

                pq = _proj(wq_sb, "q")
                rawq, rotq = _rope(pq, "q")
                pk = _proj(wk_sb, "k")
                _combine(rawq, rotq, qT, "q")
                rawk, rotk = _rope(pk, "k")
                _combine(rawk, rotk, kT, "k")

                # V^T direct: out[token, feature] = x_blk^T @ Wv_chunk
                pvt_ = pp.tile([128, 512], F32, tag="pp", name="vt")
                for tb in range(4):
                    for ko in range(8):
                        nc.tensor.matmul(
                            pvt_[:, 128 * tb:128 * tb + 128],
                            xt[:, ko, 128 * tb:128 * tb + 128],
                            wv_sb[:, ko, :],
                            start=(ko == 0), stop=(ko == 7))
                for tb in range(4):
                    jg = (t0 % T) // 128 + tb
                    for h in range(HPC):
                        nc.vector.tensor_copy(
                            vext[:, bb * HPC + h, jg, 0:64],
                            pvt_[:, 128 * tb + 64 * h:128 * tb + 64 * h + 64])

            # ============ attention for one (batch, 512-wide tq chunk)
            def attn_chunk(bb, c):
                boff = bb * T
                q0 = boff + 512 * c
                jmax = 4 * c + 4
                pvt = [pvp.tile([128, 512], F32, tag="pv",
                                name=f"pv_{bb}_{c}_{h}")
                       for h in range(HPC)]
                for j in range(jmax):
                    lo = max(0, 128 * (j - 4 * c))
                    spt = []
                    for h in range(HPC):
                        s = pp.tile([128, 512], F32, tag="pp",
                                    name=f"s_{bb}_{c}_{j}_{h}")
                        nc.tensor.matmul(
                            s[:, lo:512],
                            kT[64 * h:64 * h + 64,
                               boff + 128 * j:boff + 128 * j + 128],
                            qT[64 * h:64 * h + 64, q0 + lo:q0 + 512],
                            start=True, stop=True)
                        spt.append(s)
                    exs = []
                    for h in range(HPC):
                        ex = expp.tile([128, 512], BF16, tag="e",
                                       name=f"e_{h}")
                        nc.scalar.activation(
                            ex[:, lo:512], spt[h][:, lo:512],
                            mybir.ActivationFunctionType.Exp, scale=0.125)
                        exs.append(ex)
                    if j >= 4 * c:  # diagonal block: causal mask
                        for h in range(HPC):
                            nc.vector.tensor_tensor(
                                exs[h][:, lo:lo + 128],
                                exs[h][:, lo:lo + 128],
                                tri_sb[:], mybir.AluOpType.mult)
                    for h in range(HPC):
                        nc.tensor.matmul(
                            pvt[h][:, lo:512],
                            vext[:, bb * HPC + h, j, :],
                            exs[h][:, lo:512],
                            start=(j == 0), stop=(j == jmax - 1))
                # normalize + stage for A2A
                for h in range(HPC):
                    unn = outp.tile([64, 512], BF16, tag="unn")
                    nc.vector.tensor_copy(unn[:], pvt[h][0:64, :])
                    dn0 = stage.tile([1, 512], F32, tag="dn0",
                                     name=f"dn0_{bb}_{c}_{h}")
                    nc.vector.tensor_copy(dn0[:], pvt[h][64:65, :])
                    rec = stage.tile([1, 512], F32, tag="rec",
                                     name=f"rec_{bb}_{c}_{h}")
                    nc.vector.reciprocal_approx_fast(rec[:], dn0[:])
                    recb = outp.tile([64, 512], F32, tag="recb")
                    nc.gpsimd.partition_broadcast(recb[:], rec[:])
                    ao = outp.tile([64, 512], BF16, tag="ao")
                    nc.vector.tensor_tensor(ao[:], unn[:], recb[:],
                                            mybir.AluOpType.mult)
                    for half in range(2):
                        nc.sync.dma_start(
                            a2a_in[bb][2 * c + half,
                                       64 * h:64 * h + 64, :],
                            ao[:, 256 * half:256 * half + 256])

            # ============ output projection for one batch (post-A2A)
            def oproj(bb, oall_b):
                for eo in range(2):
                    for tb in range(2):
                        ot = pp.tile([128, 512], F32, tag="pp",
                                     name=f"ot_{bb}_{eo}_{tb}")
                        for ec in range(8):
                            nc.tensor.matmul(
                                ot, oall_b[:, ec, 128 * tb:128 * tb + 128],
                                wo_sb[:, ec, 512 * eo:512 * eo + 512],
                                start=(ec == 0), stop=(ec == 7))
                        ys = outp.tile([128, 512], F32, tag="ys")
                        nc.scalar.copy(ys[:], ot)
                        nc.sync.dma_start(
                            y_d[TPB * bb + 128 * tb:TPB * bb + 128 * tb + 128,
                                512 * eo:512 * eo + 512], ys[:])

            # ============ schedule =========================
            for ci in range(4):            # prologue: batch-0 projections
                proj_chunk(ci)
            for bb in range(B):
                for c in range(4):
                    attn_chunk(bb, c)
                    if bb + 1 < B:
                        proj_chunk(4 * (bb + 1) + c)
                nc.gpsimd.collective_compute(
                    "AllToAll", mybir.AluOpType.bypass, replica_groups=rg,
                    ins=[a2a_in[bb].opt()], outs=[a2a_out[bb].opt()])
                oall_b = oallp.tile([128, 8, TPB], BF16, tag="oall")
                nc.sync.dma_start(oall_b[:],
                                  a2a_out[bb][:].rearrange("s p t -> p s t"))
                oproj(bb, oall_b)

    nc.compile()
    return nc


def _host_inputs(x, Wq, Wk, Wv, Wo, token_positions):
    """Per-core in_maps with transposed/tiled layouts."""
    x = np.asarray(x, dtype=np.float32)
    xt_bf = np.ascontiguousarray(
        x.reshape(NT, D).T.reshape(8, 128, NT).transpose(1, 0, 2)
    ).astype(ml_dtypes.bfloat16)

    pos = np.asarray(token_positions).astype(np.float64)
    inv_freq = 1.0 / (THETA ** (np.arange(0, DH, 2, dtype=np.float64) / DH))
    ang = pos[None, :] * inv_freq[:, None]          # [32, T]
    cos_p = np.cos(ang)
    sin_p = np.sin(ang)
    d_idx = (np.arange(128) % 64) // 2
    cosb = cos_p[d_idx, :].astype(np.float32)
    sinb = sin_p[d_idx, :].astype(np.float32)

    rotm = np.zeros((128, 128), dtype=np.float32)
    for i in range(64):
        rotm[2 * i + 1, 2 * i] = -1.0
        rotm[2 * i, 2 * i + 1] = 1.0
    rotm = rotm.astype(ml_dtypes.bfloat16)
    tri = np.tril(np.ones((128, 128), dtype=np.float32)).T  # [tk, tq] tk<=tq
    tri = tri.astype(ml_dtypes.bfloat16)

    def wtiles(W, sl):
        Wt = np.ascontiguousarray(np.asarray(W, np.float32)[sl, :].T)
        return np.ascontiguousarray(
            Wt.reshape(8, 128, Wt.shape[1]).transpose(1, 0, 2)
        ).astype(ml_dtypes.bfloat16)

    WoT = np.ascontiguousarray(np.asarray(Wo, dtype=np.float32).T)
    wo_t = np.ascontiguousarray(
        WoT.reshape(8, 128, D).transpose(1, 0, 2)).astype(ml_dtypes.bfloat16)

    in_maps = []
    for c in range(N_CORES):
        sl = slice(EC * c, EC * (c + 1))
        in_maps.append({
            "xt": xt_bf,
            "wq": wtiles(Wq, sl),
            "wk": wtiles(Wk, sl),
            "wv": wtiles(Wv, sl),
            "wo": wo_t,
            "cosb": cosb,
            "sinb": sinb,
            "rotm": rotm,
            "trimask": tri,
        })
    return in_maps


def kernel(x, Wq, Wk, Wv, Wo, token_positions):
    global last_results
    if "nc" not in _CACHE:
        _CACHE["nc"] = _build_program()
    nc = _CACHE["nc"]
    in_maps = _host_inputs(x, Wq, Wk, Wv, Wo, token_positions)
    res = bass_utils.run_bass_kernel_spmd(nc, in_maps, list(range(N_CORES)))
    last_results = res
    y = np.empty((NT, D), dtype=np.float32)
    for c in range(N_CORES):
        yc = res.results[c]["y"]          # [B*TPB, D]
        for b in range(B):
            y[b * T + TPB * c:b * T + TPB * (c + 1)] = \
                yc[TPB * b:TPB * (b + 1)]
    return y.reshape(B, T, D)


# revision 9
# speedup vs baseline: 1.3488x; 1.2526x over previous
"""Multi-head self-attention (RoPE, causal) on 8 Trainium2 NeuronCores.

Sharding: tensor-parallel over heads. Each core owns 2 of 16 heads:
  - QKV projections column-sharded (each core computes its 128 features)
  - V^T computed directly (x-block stationary) so no PE transposes
  - attention per (batch, head) pair on-core, scores kept transposed
    [tk, tq]; the two heads' K=64 score matmuls run concurrently in the
    PE array (row tiling via base_partition 0/64); softmax denominator
    via a ones-column appended to V^T
  - per-batch AllToAll switches from head- to token-sharding; the output
    projection for batch b overlaps attention of batch b+1
  - emission order interleaves proj(b+1) and oproj(b-1) into attn(b) so
    the PE never idles past the HAM window (stays at 2.4 GHz)

dtypes: bf16 operands everywhere (FWL stays enabled), fp32 PSUM accum.
"""

import numpy as np
import ml_dtypes

import concourse.bacc as bacc
import concourse.mybir as mybir
import concourse.tile as tile
from concourse import bass_utils

F32 = mybir.dt.float32
BF16 = mybir.dt.bfloat16

B, T, D = 4, 2048, 1024
H, DH = 16, 64
N_CORES = 8
HPC = H // N_CORES            # heads per core = 2
EC = HPC * DH                 # feature slice per core = 128
NT = B * T                    # 8192 tokens
THETA = 10000.0
NB = T // 128                 # 16 tk blocks per batch
TPB = T // N_CORES            # 256 tokens per (core, batch) after A2A

_CACHE = {}
last_results = None


def _build_program():
    nc = bacc.Bacc("TRN2", debug=False, target_bir_lowering=False,
                   num_devices=N_CORES)

    xt_d = nc.dram_tensor("xt", [128, 8, NT], BF16, kind="ExternalInput")
    wq_d = nc.dram_tensor("wq", [128, 8, EC], BF16, kind="ExternalInput")
    wk_d = nc.dram_tensor("wk", [128, 8, EC], BF16, kind="ExternalInput")
    wv_d = nc.dram_tensor("wv", [128, 8, EC], BF16, kind="ExternalInput")
    wo_d = nc.dram_tensor("wo", [128, 8, D], BF16, kind="ExternalInput")
    cos_d = nc.dram_tensor("cosb", [128, T], F32, kind="ExternalInput")
    sin_d = nc.dram_tensor("sinb", [128, T], F32, kind="ExternalInput")
    rotm_d = nc.dram_tensor("rotm", [128, 128], BF16, kind="ExternalInput")
    tri_d = nc.dram_tensor("trimask", [128, 128], BF16, kind="ExternalInput")
    y_d = nc.dram_tensor("y", [B * TPB, D], F32, kind="ExternalOutput")

    with tile.TileContext(nc) as tc:
        with (
            tc.tile_pool(name="consts", bufs=1) as consts,
            tc.tile_pool(name="big", bufs=1) as big,
            tc.tile_pool(name="xp", bufs=2) as xp,
            tc.tile_pool(name="stage", bufs=2) as stage,
            tc.tile_pool(name="expp", bufs=6) as expp,
            tc.tile_pool(name="outp", bufs=3) as outp,
            tc.tile_pool(name="oall", bufs=2) as oallp,
            tc.tile_pool(name="pp", bufs=5, space="PSUM") as pp,
            tc.tile_pool(name="pv", bufs=3, space="PSUM") as pvp,
            tc.tile_pool(name="dram", bufs=1, space="DRAM") as dram,
        ):
            # ---- constants ----
            cos_sb = consts.tile([128, T], F32)
            sin_sb = consts.tile([128, T], F32)
            rotm_sb = consts.tile([128, 128], BF16)
            tri_sb = consts.tile([128, 128], BF16)
            nc.sync.dma_start(cos_sb[:], cos_d[:, :])
            nc.sync.dma_start(sin_sb[:], sin_d[:, :])
            nc.sync.dma_start(rotm_sb[:], rotm_d[:, :])
            nc.sync.dma_start(tri_sb[:], tri_d[:, :])

            wq_sb = consts.tile([128, 8, EC], BF16)
            wk_sb = consts.tile([128, 8, EC], BF16)
            wv_sb = consts.tile([128, 8, EC], BF16)
            wo_sb = consts.tile([128, 8, D], BF16)
            nc.sync.dma_start(wq_sb[:], wq_d[:, :, :])
            nc.sync.dma_start(wk_sb[:], wk_d[:, :, :])
            nc.sync.dma_start(wv_sb[:], wv_d[:, :, :])
            nc.sync.dma_start(wo_sb[:], wo_d[:, :, :])

            # ---- persistent tensors ----
            qT = big.tile([128, NT], BF16, tag="qT")
            kT = big.tile([128, NT], BF16, tag="kT")
            # V^T per (pair, tk-block): [tk=128, 128] cols 0-63 = v,
            # col 64 = ones (softmax denominator), cols 65-127 unused
            # (full-128 stationary keeps FWL enabled on the PV matmuls).
            vext = big.tile([128, HPC * B, NB, 128], BF16, tag="vext")
            nc.vector.memset(vext[:, :, :, 64], 1.0)

            a2a_in = [dram.tile([N_CORES, 128, TPB], BF16, tag=f"a2ai{b}",
                                name=f"a2ai{b}") for b in range(B)]
            a2a_out = [dram.tile([N_CORES, 128, TPB], BF16, tag=f"a2ao{b}",
                                 name=f"a2ao{b}") for b in range(B)]
            rg = [list(range(N_CORES))]

            # ============ QKV projections + RoPE for one 512-token chunk,
            # sliced into small PE quanta so they weave between attention
            # j-iterations without starving the ACT exp stream.
            def proj_chunk_steps(ci):
                t0 = 512 * ci
                bb = t0 // T
                state = {}

                def load_x():
                    xt = xp.tile([128, 8, 512], BF16, tag="x",
                                 name=f"x_{ci}")
                    nc.sync.dma_start(xt[:], xt_d[:, :, t0:t0 + 512])
                    state["xt"] = xt

                def _proj(w_sb, nm):
                    pt = pp.tile([128, 512], F32, tag="pp",
                                 name=f"p{nm}_{ci}")
                    for ko in range(8):
                        nc.tensor.matmul(pt, w_sb[:, ko, :],
                                         state["xt"][:, ko, :],
                                         start=(ko == 0), stop=(ko == 7))
                    state["p" + nm] = pt

                def _rope(nm):
                    raw = stage.tile([128, 512], BF16, tag="raw" + nm,
                                     name=f"raw{nm}_{ci}")
                    nc.vector.tensor_copy(raw[:], state["p" + nm])
                    rot = pp.tile([128, 512], F32, tag="pp",
                                  name=f"r{nm}_{ci}")
                    nc.tensor.matmul(rot, rotm_sb[:], raw[:],
                                     start=True, stop=True)
                    state["raw" + nm] = raw
                    state["rot" + nm] = rot

                def _combine(nm, dest):
                    s0 = t0 % T
                    raw, rot = state["raw" + nm], state["rot" + nm]
                    t1 = stage.tile([128, 512], F32, tag="t1" + nm,
                                    name=f"t1{nm}_{ci}")
                    nc.vector.tensor_tensor(
                        t1[:], raw[:], cos_sb[:, s0:s0 + 512],
                        mybir.AluOpType.mult)
                    t2 = stage.tile([128, 512], F32, tag="t2" + nm,
                                    name=f"t2{nm}_{ci}")
                    nc.vector.tensor_tensor(
                        t2[:], rot[:], sin_sb[:, s0:s0 + 512],
                        mybir.AluOpType.mult)
                    nc.vector.tensor_tensor(
                        dest[:, t0:t0 + 512], t1[:], t2[:],
                        mybir.AluOpType.add)

                def _vt(tb):
                    if tb == 0:
                        state["vt"] = pp.tile([128, 512], F32, tag="pp",
                                              name=f"vt_{ci}")
                    pvt_ = state["vt"]
                    for ko in range(8):
                        nc.tensor.matmul(
                            pvt_[:, 128 * tb:128 * tb + 128],
                            state["xt"][:, ko, 128 * tb:128 * tb + 128],
                            wv_sb[:, ko, :],
                            start=(ko == 0), stop=(ko == 7))

                def _vcopy(tb):
                    jg = (t0 % T) // 128 + tb
                    for h in range(HPC):
                        nc.vector.tensor_copy(
                            vext[:, bb * HPC + h, jg, 0:64],
                            state["vt"][:, 128 * tb + 64 * h:
                                        128 * tb + 64 * h + 64])

                return [
                    load_x,
                    lambda: _proj(wq_sb, "q"),
                    lambda: _rope("q"),
                    lambda: (_combine("q", qT), _proj(wk_sb, "k")),
                    lambda: _rope("k"),
                    lambda: _combine("k", kT),
                    lambda: _vt(0),
                    lambda: _vt(1),
                    lambda: (_vcopy(0), _vt(2)),
                    lambda: (_vcopy(1), _vt(3)),
                    lambda: (_vcopy(2), _vcopy(3)),
                ]

            # ============ attention for one (batch, 512-wide tq chunk)
            def attn_chunk(bb, c, steps):
                boff = bb * T
                q0 = boff + 512 * c
                jmax = 4 * c + 4
                pvt = [pvp.tile([128, 512], F32, tag="pv",
                                name=f"pv_{bb}_{c}_{h}")
                       for h in range(HPC)]
                for j in range(jmax):
                    lo = max(0, 128 * (j - 4 * c))
                    spt = []
                    for h in range(HPC):
                        s = pp.tile([128, 512], F32, tag="pp",
                                    name=f"s_{bb}_{c}_{j}_{h}")
                        nc.tensor.matmul(
                            s[:, lo:512],
                            kT[64 * h:64 * h + 64,
                               boff + 128 * j:boff + 128 * j + 128],
                            qT[64 * h:64 * h + 64, q0 + lo:q0 + 512],
                            start=True, stop=True)
                        spt.append(s)
                    exs = []
                    for h in range(HPC):
                        ex = expp.tile([128, 512], BF16, tag="e",
                                       name=f"e_{h}")
                        nc.scalar.activation(
                            ex[:, lo:512], spt[h][:, lo:512],
                            mybir.ActivationFunctionType.Exp, scale=0.125)
                        exs.append(ex)
                    if j >= 4 * c:  # diagonal block: causal mask
                        for h in range(HPC):
                            nc.vector.tensor_tensor(
                                exs[h][:, lo:lo + 128],
                                exs[h][:, lo:lo + 128],
                                tri_sb[:], mybir.AluOpType.mult)
                    for h in range(HPC):
                        nc.tensor.matmul(
                            pvt[h][:, lo:512],
                            vext[:, bb * HPC + h, j, :],
                            exs[h][:, lo:512],
                            start=(j == 0), stop=(j == jmax - 1))
                    if steps:
                        steps.popleft()()
                # normalize + stage for A2A
                for h in range(HPC):
                    unn = outp.tile([64, 512], BF16, tag="unn")
                    nc.vector.tensor_copy(unn[:], pvt[h][0:64, :])
                    dn0 = stage.tile([1, 512], F32, tag="dn0",
                                     name=f"dn0_{bb}_{c}_{h}")
                    nc.vector.tensor_copy(dn0[:], pvt[h][64:65, :])
                    rec = stage.tile([1, 512], F32, tag="rec",
                                     name=f"rec_{bb}_{c}_{h}")
                    nc.vector.reciprocal_approx_fast(rec[:], dn0[:])
                    recb = outp.tile([64, 512], F32, tag="recb")
                    nc.gpsimd.partition_broadcast(recb[:], rec[:])
                    ao = outp.tile([64, 512], BF16, tag="ao")
                    nc.vector.tensor_tensor(ao[:], unn[:], recb[:],
                                            mybir.AluOpType.mult)
                    for half in range(2):
                        nc.sync.dma_start(
                            a2a_in[bb][2 * c + half,
                                       64 * h:64 * h + 64, :],
                            ao[:, 256 * half:256 * half + 256])

            # ============ output projection for one batch (post-A2A)
            def oproj_steps(bb):
                state = [None]

                def first():
                    oall_b = oallp.tile([128, 8, TPB], BF16, tag="oall",
                                        name=f"oall_{bb}")
                    nc.sync.dma_start(
                        oall_b[:], a2a_out[bb][:].rearrange("s p t -> p s t"))
                    state[0] = oall_b

                def one(eo, tb):
                    oall_b = state[0]
                    ot = pp.tile([128, 512], F32, tag="pp",
                                 name=f"ot_{bb}_{eo}_{tb}")
                    for ec in range(8):
                        nc.tensor.matmul(
                            ot, oall_b[:, ec, 128 * tb:128 * tb + 128],
                            wo_sb[:, ec, 512 * eo:512 * eo + 512],
                            start=(ec == 0), stop=(ec == 7))
                    ys = outp.tile([128, 512], F32, tag="ys")
                    nc.scalar.copy(ys[:], ot)
                    nc.sync.dma_start(
                        y_d[TPB * bb + 128 * tb:TPB * bb + 128 * tb + 128,
                            512 * eo:512 * eo + 512], ys[:])

                return [first,
                        lambda: one(0, 0), lambda: one(0, 1),
                        lambda: one(1, 0), lambda: one(1, 1)]

            # ============ schedule =========================
            from collections import deque
            for ci in range(4):            # prologue: batch-0 projections
                for s in proj_chunk_steps(ci):
                    s()
            steps = deque()
            for bb in range(B):
                if bb + 1 < B:
                    for ci in range(4 * (bb + 1), 4 * (bb + 1) + 4):
                        steps.extend(proj_chunk_steps(ci))
                if bb >= 1:
                    steps.extend(oproj_steps(bb - 1))
                for c in range(4):
                    attn_chunk(bb, c, steps)
                while steps:
                    steps.popleft()()
                nc.gpsimd.collective_compute(
                    "AllToAll", mybir.AluOpType.bypass, replica_groups=rg,
                    ins=[a2a_in[bb].opt()], outs=[a2a_out[bb].opt()])
            for s in oproj_steps(B - 1):   # tail
                s()

    nc.compile()
    return nc


def _host_inputs(x, Wq, Wk, Wv, Wo, token_positions):
    """Per-core in_maps with transposed/tiled layouts."""
    x = np.asarray(x, dtype=np.float32)
    xt_bf = np.ascontiguousarray(
        x.reshape(NT, D).T.reshape(8, 128, NT).transpose(1, 0, 2)
    ).astype(ml_dtypes.bfloat16)

    pos = np.asarray(token_positions).astype(np.float64)
    inv_freq = 1.0 / (THETA ** (np.arange(0, DH, 2, dtype=np.float64) / DH))
    ang = pos[None, :] * inv_freq[:, None]          # [32, T]
    cos_p = np.cos(ang)
    sin_p = np.sin(ang)
    d_idx = (np.arange(128) % 64) // 2
    cosb = cos_p[d_idx, :].astype(np.float32)
    sinb = sin_p[d_idx, :].astype(np.float32)

    rotm = np.zeros((128, 128), dtype=np.float32)
    for i in range(64):
        rotm[2 * i + 1, 2 * i] = -1.0
        rotm[2 * i, 2 * i + 1] = 1.0
    rotm = rotm.astype(ml_dtypes.bfloat16)
    tri = np.tril(np.ones((128, 128), dtype=np.float32)).T  # [tk, tq] tk<=tq
    tri = tri.astype(ml_dtypes.bfloat16)

    def wtiles(W, sl):
        Wt = np.ascontiguousarray(np.asarray(W, np.float32)[sl, :].T)
        return np.ascontiguousarray(
            Wt.reshape(8, 128, Wt.shape[1]).transpose(1, 0, 2)
        ).astype(ml_dtypes.bfloat16)

    WoT = np.ascontiguousarray(np.asarray(Wo, dtype=np.float32).T)
    wo_t = np.ascontiguousarray(
        WoT.reshape(8, 128, D).transpose(1, 0, 2)).astype(ml_dtypes.bfloat16)

    in_maps = []
    for c in range(N_CORES):
        sl = slice(EC * c, EC * (c + 1))
        in_maps.append({
            "xt": xt_bf,
            "wq": wtiles(Wq, sl),
            "wk": wtiles(Wk, sl),
            "wv": wtiles(Wv, sl),
            "wo": wo_t,
            "cosb": cosb,
            "sinb": sinb,
            "rotm": rotm,
            "trimask": tri,
        })
    return in_maps


def kernel(x, Wq, Wk, Wv, Wo, token_positions):
    global last_results
    if "nc" not in _CACHE:
        _CACHE["nc"] = _build_program()
    nc = _CACHE["nc"]
    in_maps = _host_inputs(x, Wq, Wk, Wv, Wo, token_positions)
    res = bass_utils.run_bass_kernel_spmd(nc, in_maps, list(range(N_CORES)))
    last_results = res
    y = np.empty((NT, D), dtype=np.float32)
    for c in range(N_CORES):
        yc = res.results[c]["y"]          # [B*TPB, D]
        for b in range(B):
            y[b * T + TPB * c:b * T + TPB * (c + 1)] = \
                yc[TPB * b:TPB * (b + 1)]
    return y.reshape(B, T, D)
